# revision 1
# baseline (speedup 1.0000x reference)
"""Trainium2 Bass kernel for nn_DeepGATEncoder (3-layer GAT + mean-pool + MLP).

Sharding: the 3072 nodes' attention rows are split 384/core across 8 cores.
Weights are replicated. Each core computes Wh (+ attention-bias column d and a
ones column for the softmax denominator) for its own 384 nodes per head, the
per-head blocks are AllGather'ed, and each core then runs masked softmax
attention for its own rows against all 3072 columns. Between layers only the
locally-owned columns of h^T are needed, so no further gathers. The mean-pool
partials are AllReduce'd and the tiny MLP is computed redundantly on every
core.

Softmax is computed without max-subtraction (safe: e = lrelu(s_i+d_j) stays
< ~15 for glorot-initialized weights at these widths, far below fp32 exp
overflow) via
    exp(lrelu(z)) = max(exp(z), exp(.02 z)),  exp(.02 z) = exp(.02 s)exp(.02 d)
so the inner loop is one ACT exp + three cheap DVE/GPSIMD ops per 128x384
tile, and the softmax denominator falls out of the attention matmul via an
appended ones column.
"""

import os
import numpy as np

import concourse.bass as bass
import concourse.bacc as bacc
import concourse.mybir as mybir
import concourse.tile as tile
from concourse.bass_utils import run_bass_kernel_spmd

# ---- problem constants (hardcoded; kernel.py must be self-contained) ----
N = 3072
F_IN = 300
HID = 300
OUT_ATT = 600
HEADS = 10
N_GRAPHS = 96
MLP_HID = 600
NOUT = 768
ALPHA = 0.02

NCORES = 8
RPC = N // NCORES          # 384 rows (nodes) per core
NJT = RPC // 128           # 3 own-row tiles of 128
NCH = N // 128             # 24 column chunks of 128

F32 = mybir.dt.float32
BF16 = mybir.dt.bfloat16
AF = mybir.ActivationFunctionType

# compute dtype for matmul operands / streamed data
CDT = F32 if os.environ.get("KERNEL_F32") else BF16

TRACE = bool(os.environ.get("KERNEL_TRACE"))
DEBUG_STAGE = os.environ.get("KERNEL_DEBUG", "")

_compiled = {}


class _EarlyExit(Exception):
    pass


def _chunks(total, step=128):
    out = []
    lo = 0
    while lo < total:
        out.append((lo, min(step, total - lo)))
        lo += step
    return out


def _mm(nc, out, lhsT, rhs, **kw):
    """matmul, with float32r (full-rate fp32) when operands are fp32."""
    if lhsT.dtype == F32:
        lhsT = lhsT.bitcast(mybir.dt.float32r)
        rhs = rhs.bitcast(mybir.dt.float32r)
    nc.tensor.matmul(out, lhsT, rhs, **kw)


def build():
    nc = bacc.Bacc("TRN2", target_bir_lowering=False, debug=False,
                   num_devices=NCORES)

    # ---------------- external I/O (per-core data) ----------------
    xT = nc.dram_tensor("xT", [F_IN, RPC], CDT, kind="ExternalInput")
    adjT = nc.dram_tensor("adjT", [N, RPC], CDT, kind="ExternalInput")
    smat = nc.dram_tensor("smat", [RPC, N_GRAPHS], CDT, kind="ExternalInput")
    # per-layer fused weights: [heads, F, 301] with col 300 = W @ a_dst
    R0 = nc.dram_tensor("R0", [HEADS, F_IN, HID + 1], CDT, kind="ExternalInput")
    WA0 = nc.dram_tensor("WA0", [F_IN, HEADS], CDT, kind="ExternalInput")
    R1 = nc.dram_tensor("R1", [HEADS, HEADS * HID, HID + 1], CDT, kind="ExternalInput")
    WA1 = nc.dram_tensor("WA1", [HEADS * HID, HEADS], CDT, kind="ExternalInput")
    Ro = nc.dram_tensor("Ro", [HEADS * HID, OUT_ATT + 1], CDT, kind="ExternalInput")
    WAo = nc.dram_tensor("WAo", [HEADS * HID, 1], CDT, kind="ExternalInput")
    Wm1 = nc.dram_tensor("Wm1", [OUT_ATT, MLP_HID], CDT, kind="ExternalInput")
    bm1 = nc.dram_tensor("bm1", [MLP_HID, 1], F32, kind="ExternalInput")
    Wm2 = nc.dram_tensor("Wm2", [MLP_HID, NOUT], CDT, kind="ExternalInput")
    bm2 = nc.dram_tensor("bm2", [NOUT, 1], F32, kind="ExternalInput")
    eye10f = nc.dram_tensor("eye10f", [HEADS, HEADS * 128], F32, kind="ExternalInput")
    eye10c = nc.dram_tensor("eye10c", [HEADS, HEADS * 128], CDT, kind="ExternalInput")
    outT = nc.dram_tensor("outT", [NOUT, N_GRAPHS], F32, kind="ExternalOutput")
    dbg = None
    if DEBUG_STAGE in ("L0", "L1"):
        dbg = nc.dram_tensor("dbg", [HEADS * HID, RPC], F32, kind="ExternalOutput")
    elif DEBUG_STAGE == "S0":
        dbg = nc.dram_tensor("dbg", [HEADS, RPC], F32, kind="ExternalOutput")
    elif DEBUG_STAGE == "WG0":
        dbg = nc.dram_tensor("dbg", [N, HID + 1], F32, kind="ExternalOutput")
    elif DEBUG_STAGE == "HO":
        dbg = nc.dram_tensor("dbg", [RPC, OUT_ATT], F32, kind="ExternalOutput")
    elif DEBUG_STAGE == "POOL":
        dbg = nc.dram_tensor("dbg", [OUT_ATT, N_GRAPHS], F32, kind="ExternalOutput")

    rg = [list(range(NCORES))]

    with tile.TileContext(nc) as tc:
        with (
            tc.tile_pool(name="persist", bufs=1) as persist,
            tc.tile_pool(name="whbuf", bufs=2) as whbufp,
            tc.tile_pool(name="rstream", bufs=4) as rstream,
            tc.tile_pool(name="ew", bufs=3) as ew,
            tc.tile_pool(name="small", bufs=2) as small,
            tc.tile_pool(name="ps", bufs=1, space="PSUM") as ps,
            tc.tile_pool(name="dram", bufs=1, space="DRAM") as dram,
        ):
            # ---------- persistent SBUF state ----------
            adj_sb = persist.tile([128, NCH * RPC], CDT, name="adj_sb")
            nc.sync.dma_start(adj_sb[:].rearrange("p (c i) -> p c i", i=RPC),
                              adjT[:].rearrange("(c p) i -> p c i", p=128))

            smat_sb = [persist.tile([128, N_GRAPHS], CDT, name=f"smat{i}")
                       for i in range(NJT)]
            for i in range(NJT):
                nc.sync.dma_start(smat_sb[i][:], smat[i * 128:(i + 1) * 128, :])

            eyef_sb = persist.tile([HEADS, HEADS * 128], F32, name="eyef_sb")
            nc.sync.dma_start(eyef_sb[:], eye10f[:])
            eyec_sb = persist.tile([HEADS, HEADS * 128], CDT, name="eyec_sb")
            nc.sync.dma_start(eyec_sb[:], eye10c[:])

            # h^T storage between layers (own columns only), [3000, 384]
            hT = dram.tile([HEADS * HID, RPC], CDT, name="hT")

            def load_ht(src_loader, kch):
                tiles = []
                for ci, (lo, sz) in enumerate(kch):
                    t = rstream.tile([128, RPC], CDT, tag=f"ht_{ci}", bufs=1,
                                     name=f"ht_{ci}", uniquify=True)
                    src_loader(t, lo, sz)
                    tiles.append(t)
                return tiles

            onesf_sb = persist.tile([1, 128], F32, name="onesf_sb")
            nc.vector.memset(onesf_sb[:], 1.0)
            onesc_sb = persist.tile([1, 128], CDT, name="onesc_sb")
            nc.vector.memset(onesc_sb[:], 1.0)
            s_sb = persist.tile([HEADS, RPC], F32, name="s_sb")
            es02_sb = persist.tile([HEADS, RPC], CDT, name="es02_sb")
            sbc_all = persist.tile([128, HEADS * RPC], F32, name="sbc_all")
            e02bc_all = persist.tile([128, HEADS * RPC], CDT, name="e02bc_all")

            # ============ one GAT layer ============
            def gat_layer(lidx, fin, r_dram, wa_dram, ht_loader):
                kch = _chunks(fin)
                nkc = len(kch)
                ht_sb = load_ht(ht_loader, kch)

                # --- s vector for all heads: [heads, own-i] ---
                ps_s = ps.tile([HEADS, RPC], F32, tag="p6")
                for ci, (lo, sz) in enumerate(kch):
                    wa_t = rstream.tile([128, HEADS], CDT, tag="wa", bufs=6)
                    nc.sync.dma_start(wa_t[:sz, :], wa_dram[lo:lo + sz, :])
                    _mm(nc, ps_s[:], wa_t[:sz, :], ht_sb[ci][:sz, :],
                        start=(ci == 0), stop=(ci == nkc - 1))
                nc.scalar.activation(s_sb[:], ps_s[:], AF.Copy)
                nc.scalar.activation(es02_sb[:], ps_s[:], AF.Exp, scale=ALPHA)
                for h in range(HEADS):
                    pb = ps.tile([128, RPC], F32, tag="p6", name=f"pb{h}")
                    nc.tensor.matmul(pb[:], eyef_sb[:, h * 128:(h + 1) * 128],
                                     s_sb[:], start=True, stop=True)
                    nc.scalar.activation(sbc_all[:, h * RPC:(h + 1) * RPC],
                                         pb[:], AF.Copy)
                    pb2 = ps.tile([128, RPC], F32, tag="p6", name=f"pb2{h}")
                    nc.tensor.matmul(pb2[:], eyec_sb[:, h * 128:(h + 1) * 128],
                                     es02_sb[:], start=True, stop=True)
                    nc.scalar.activation(e02bc_all[:, h * RPC:(h + 1) * RPC],
                                         pb2[:], AF.Copy)

                wg_list = []
                # --- per-head: local Wh -> DRAM -> AllGather ---
                for h in range(HEADS):
                    wl = dram.tile([RPC, HID + 2], CDT, name=f"wl{lidx}_{h}")
                    wg = dram.tile([N, HID + 2], CDT, name=f"wg{lidx}_{h}", addr_space="Shared")
                    wg_list.append(wg)
                    wl_sb = small.tile([128, NJT * (HID + 2)], CDT, tag="wl_sb")
                    psw_t = [ps.tile([128, HID + 1], F32, tag=f"p{jt}", name=f"psw{jt}")
                             for jt in range(NJT)]
                    for ci, (lo, sz) in enumerate(kch):
                        r_t = rstream.tile([128, HID + 1], CDT, tag="r", bufs=8)
                        nc.sync.dma_start(r_t[:sz, :], r_dram[h, lo:lo + sz, :])
                        for jt in range(NJT):
                            _mm(nc, psw_t[jt][:],
                                ht_sb[ci][:sz, jt * 128:(jt + 1) * 128],
                                r_t[:sz, :],
                                start=(ci == 0), stop=(ci == nkc - 1))
                    for jt in range(NJT):
                        base = jt * (HID + 2)
                        nc.vector.memset(wl_sb[:, base:base + 1], 1.0)
                        nc.scalar.activation(wl_sb[:, base + 1:base + HID + 2],
                                             psw_t[jt][:], AF.Copy)
                        nc.scalar.dma_start(wl[jt * 128:(jt + 1) * 128, :],
                                            wl_sb[:, base:base + HID + 2])
                    nc.gpsimd.collective_compute(
                        "AllGather", mybir.AluOpType.bypass, replica_groups=rg,
                        ins=[wl.opt()], outs=[wg.opt()])

                # --- per-head attention over gathered Wh ---
                # Software-pipelined: head h's divide/ELU/store is emitted
                # after head h+1's matmul phase so PE never starves at head
                # boundaries.
                W2 = HID + 2
                osp = [(0, 128), (128, 128), (256, 45)]

                def att_front(h):
                    wg = wg_list[h]
                    whb = whbufp.tile([128, NCH * W2], CDT, tag="whb",
                                      name=f"whb{h}")
                    nc.sync.dma_start(
                        whb[:].rearrange("p (c w) -> p c w", w=W2),
                        wg[:].rearrange("(c p) w -> p c w", p=128))
                    ed02 = small.tile([128, NCH], F32, tag="ed02",
                                      name=f"ed02_{h}")
                    dcols = whb[:].rearrange("p (c w) -> p c w", w=W2)[:, :, HID + 1]
                    nc.scalar.activation(ed02[:], dcols, AF.Exp, scale=ALPHA)
                    sbc = sbc_all[:, h * RPC:(h + 1) * RPC]
                    e02bc = e02bc_all[:, h * RPC:(h + 1) * RPC]

                    ps_att = [ps.tile([128, RPC], F32, tag=f"p{3 + om}",
                                      name=f"psatt{om}")
                              for om in range(len(osp))]
                    GRP = 3
                    for c0 in range(0, NCH, GRP):
                        a_t = ew.tile([128, GRP * RPC], CDT, tag="a", bufs=2)
                        b_t = ew.tile([128, GRP * RPC], CDT, tag="b", bufs=2)
                        for k in range(GRP):
                            c = c0 + k
                            nc.scalar.activation(
                                a_t[:, k * RPC:(k + 1) * RPC], sbc, AF.Exp,
                                bias=whb[:, c * W2 + HID + 1:c * W2 + HID + 2])
                            nc.vector.tensor_scalar_mul(
                                b_t[:, k * RPC:(k + 1) * RPC], e02bc,
                                ed02[:, c:c + 1])
                        nc.vector.tensor_max(b_t[:], a_t[:], b_t[:])
                        p_t = ew.tile([128, GRP * RPC], CDT, tag="p", bufs=5)
                        nc.vector.tensor_mul(
                            p_t[:], b_t[:],
                            adj_sb[:, c0 * RPC:(c0 + GRP) * RPC])
                        for k in range(GRP):
                            c = c0 + k
                            for om, (lo, sz) in enumerate(osp):
                                _mm(nc, ps_att[om][:sz, :],
                                    whb[:, c * W2 + lo:c * W2 + lo + sz],
                                    p_t[:, k * RPC:(k + 1) * RPC],
                                    start=(c == 0), stop=(c == NCH - 1))
                    # free PSUM fast: copy raw sums to SBUF
                    att32 = [ew.tile([128, RPC], F32, tag=f"att32_{om}",
                                     name=f"att32_{om}", bufs=2)
                             for om in range(len(osp))]
                    with tc.high_priority():
                        for om, (lo, sz) in enumerate(osp):
                            nc.scalar.activation(att32[om][:sz, :],
                                                 ps_att[om][:sz, :], AF.Copy)
                    return att32

                def att_back(h, att32):
                    scr = small.tile([1, RPC], F32, tag="scr")
                    rden = small.tile([1, RPC], F32, tag="rden")
                    nc.vector.reciprocal_approx_accurate(
                        rden[:], att32[0][0:1, :], scr[:])
                    rbc = ps.tile([128, RPC], F32, tag="p7", name="rbc")
                    nc.tensor.matmul(rbc[:], onesf_sb[:], rden[:],
                                     start=True, stop=True)
                    # om0 rows: 0=denominator junk, 1..127 = Wh outs 0..126
                    store = [(0, 0, 128, 1), (1, 127, 128, 0), (2, 255, 45, 0)]
                    for om, hlo, rows, roff in store:
                        y_t = ew.tile([128, RPC], CDT, tag="y")
                        nc.vector.tensor_mul(y_t[:rows, :], att32[om][:rows, :],
                                             rbc[:rows, :])
                        q_t = ew.tile([128, RPC], CDT, tag="q")
                        nc.scalar.activation(q_t[:rows, :], y_t[:rows, :], AF.Exp)
                        nc.vector.tensor_scalar(q_t[:rows, :], q_t[:rows, :],
                                                -1.0, 0.0, mybir.AluOpType.add,
                                                mybir.AluOpType.min)
                        nc.vector.tensor_max(y_t[:rows, :], y_t[:rows, :],
                                             q_t[:rows, :])
                        nrows = rows - roff
                        nc.scalar.dma_start(
                            hT[h * HID + hlo:h * HID + hlo + nrows, :],
                            y_t[roff:rows, :])

                pend = None
                for h in range(HEADS):
                    a32 = att_front(h)
                    if pend is not None:
                        att_back(*pend)
                    pend = (h, a32)
                att_back(*pend)
                return wg_list[0]

            def _tail():
                # ---------------- output attention layer ----------------
                FO = HEADS * HID
                kch = _chunks(FO)
                nkc = len(kch)
                ht_sb = load_ht(
                    lambda t, lo, sz: nc.sync.dma_start(t[:sz, :], hT[lo:lo + sz, :]),
                    kch)
                # s vector
                ps_so = ps.tile([1, RPC], F32, tag="p6")
                for ci, (lo, sz) in enumerate(kch):
                    wa_t = rstream.tile([128, 1], CDT, tag="wa", bufs=6)
                    nc.sync.dma_start(wa_t[:sz, :], WAo[lo:lo + sz, :])
                    _mm(nc, ps_so[:], wa_t[:sz, :], ht_sb[ci][:sz, :],
                        start=(ci == 0), stop=(ci == nkc - 1))
                so_sb = persist.tile([1, RPC], F32, name="so_sb")
                nc.scalar.activation(so_sb[:], ps_so[:], AF.Copy)
                eso02 = persist.tile([1, RPC], CDT, name="eso02")
                nc.scalar.activation(eso02[:], ps_so[:], AF.Exp, scale=ALPHA)

                # local Wh_out [RPC, 602] -> 3 AllGathers (one per own j-tile)
                WO2 = OUT_ATT + 2
                wlo_p = [dram.tile([128, WO2], CDT, name=f"wlo{jt}")
                         for jt in range(NJT)]
                wgo_p = [dram.tile([N // NJT, WO2], CDT, name=f"wgo{jt}",
                                   addr_space="Shared") for jt in range(NJT)]
                nsp = _chunks(OUT_ATT + 1, 512)  # [(0,512),(512,89)]
                wlo_sb = small.tile([128, NJT * WO2], CDT, tag="wlo_sb")
                for jt in range(NJT):
                    pswo = [ps.tile([128, sz], F32, tag=f"p{si}", name=f"pswo{si}")
                            for si, (lo, sz) in enumerate(nsp)]
                    for ci, (lo, sz) in enumerate(kch):
                        r_t = rstream.tile([128, OUT_ATT + 1], CDT, tag="ro")
                        nc.sync.dma_start(r_t[:sz, :], Ro[lo:lo + sz, :])
                        for si, (slo, ssz) in enumerate(nsp):
                            _mm(nc, pswo[si][:],
                                ht_sb[ci][:sz, jt * 128:(jt + 1) * 128],
                                r_t[:sz, slo:slo + ssz],
                                start=(ci == 0), stop=(ci == nkc - 1))
                    base = jt * WO2
                    for si, (slo, ssz) in enumerate(nsp):
                        nc.scalar.activation(wlo_sb[:, base + slo:base + slo + ssz],
                                             pswo[si][:], AF.Copy)
                    nc.vector.memset(wlo_sb[:, base + WO2 - 1:base + WO2], 1.0)
                    nc.scalar.dma_start(wlo_p[jt][:], wlo_sb[:, base:base + WO2])
                    nc.gpsimd.collective_compute(
                        "AllGather", mybir.AluOpType.bypass, replica_groups=rg,
                        ins=[wlo_p[jt].opt()], outs=[wgo_p[jt].opt()])

                # attention (single head), row-major output for pooling
                pbo = ps.tile([128, RPC], F32, tag="p6", name="pbo")
                nc.tensor.matmul(pbo[:], onesf_sb[:], so_sb[:],
                                 start=True, stop=True)
                sbco = ew.tile([128, RPC], F32, tag="sbc")
                nc.scalar.activation(sbco[:], pbo[:], AF.Copy)
                pbo2 = ps.tile([128, RPC], F32, tag="p6", name="pbo2")
                nc.tensor.matmul(pbo2[:], onesc_sb[:], eso02[:],
                                 start=True, stop=True)
                e02bco = ew.tile([128, RPC], CDT, tag="e02bc")
                nc.scalar.activation(e02bco[:], pbo2[:], AF.Copy)

                onsp = _chunks(OUT_ATT + 2, 512)  # [(0,512),(512,90)]
                ps_o = [[ps.tile([128, sz], F32, tag=f"p{it * 2 + si}", name=f"pso{it}_{si}")
                         for si, (lo, sz) in enumerate(onsp)] for it in range(NJT)]
                GRP = 3
                for c0 in range(0, NCH, GRP):
                    whcs = []
                    a_t = ew.tile([128, GRP * RPC], CDT, tag="a", bufs=2)
                    b_t = ew.tile([128, GRP * RPC], CDT, tag="b", bufs=2)
                    for k in range(GRP):
                        c = c0 + k
                        whc = rstream.tile([128, WO2], CDT, tag=f"whc{k}",
                                           name=f"whc{c}", bufs=2)
                        nc.sync.dma_start(
                            whc[:],
                            wgo_p[c % NJT][(c // NJT) * 128:(c // NJT + 1) * 128, :])
                        whcs.append(whc)
                        edo02c = small.tile([128, 1], F32, tag="ed02c", bufs=3)
                        nc.scalar.activation(edo02c[:], whc[:, OUT_ATT:OUT_ATT + 1],
                                             AF.Exp, scale=ALPHA)
                        nc.scalar.activation(
                            a_t[:, k * RPC:(k + 1) * RPC], sbco[:], AF.Exp,
                            bias=whc[:, OUT_ATT:OUT_ATT + 1])
                        nc.vector.tensor_scalar_mul(
                            b_t[:, k * RPC:(k + 1) * RPC], e02bco[:], edo02c[:])
                    nc.vector.tensor_max(b_t[:], a_t[:], b_t[:])
                    p_t = ew.tile([128, GRP * RPC], CDT, tag="p", bufs=5)
                    nc.vector.tensor_mul(p_t[:], b_t[:],
                                         adj_sb[:, c0 * RPC:(c0 + GRP) * RPC])
                    for k in range(GRP):
                        c = c0 + k
                        for it in range(NJT):
                            for si, (slo, ssz) in enumerate(onsp):
                                _mm(nc, ps_o[it][si][:],
                                    p_t[:, k * RPC + it * 128:k * RPC + (it + 1) * 128],
                                    whcs[k][:, slo:slo + ssz],
                                    start=(c == 0), stop=(c == NCH - 1))

                # divide + ELU + pooled^T partial via matmul with smat
                gsp = _chunks(OUT_ATT)  # 600 -> 128x4 + 88
                att_tiles = []
                for it in range(NJT):
                    denc = small.tile([128, 1], F32, tag="denc")
                    scrc = small.tile([128, 1], F32, tag="scrc")
                    rdenc = small.tile([128, 1], F32, tag="rdenc")
                    nc.vector.tensor_copy(denc[:], ps_o[it][-1][:, 89:90])
                    nc.vector.reciprocal_approx_accurate(rdenc[:], denc[:], scrc[:])
                    att_sb = ew.tile([128, OUT_ATT], CDT, tag=f"atts{it}",
                                     name=f"atts{it}", bufs=1)
                    for si, (slo, ssz) in enumerate(onsp):
                        cols = min(ssz, OUT_ATT - slo)
                        y = att_sb[:, slo:slo + cols]
                        nc.vector.tensor_scalar_mul(y, ps_o[it][si][:, :cols], rdenc[:])
                        q_t = ew.tile([128, OUT_ATT], CDT, tag="qo")
                        nc.scalar.activation(q_t[:, :cols], y, AF.Exp)
                        nc.vector.tensor_scalar(q_t[:, :cols], q_t[:, :cols],
                                                -1.0, 0.0, mybir.AluOpType.add,
                                                mybir.AluOpType.min)
                        nc.vector.tensor_max(y, y, q_t[:, :cols])
                    if DEBUG_STAGE == "HO":
                        a32 = ew.tile([128, OUT_ATT], F32, tag="a32dbg")
                        nc.vector.tensor_copy(a32[:], att_sb[:])
                        nc.sync.dma_start(dbg[it * 128:(it + 1) * 128, :], a32[:])
                    att_tiles.append(att_sb)

                # pooled^T [600, 96] -> AllReduce
                pool_l = dram.tile([OUT_ATT, N_GRAPHS], F32, name="pool_l")
                pool_g = dram.tile([OUT_ATT, N_GRAPHS], F32, name="pool_g", addr_space="Shared")
                for g, (glo, gsz) in enumerate(gsp):
                    psp = ps.tile([128, N_GRAPHS], F32, tag="p7", name=f"psp{g}")
                    for it in range(NJT):
                        _mm(nc, psp[:gsz, :], att_tiles[it][:, glo:glo + gsz],
                            smat_sb[it][:], start=(it == 0), stop=(it == NJT - 1))
                    pl_sb = small.tile([128, N_GRAPHS], F32, tag="pl_sb")
                    nc.scalar.activation(pl_sb[:gsz, :], psp[:gsz, :], AF.Copy)
                    nc.sync.dma_start(pool_l[glo:glo + gsz, :], pl_sb[:gsz, :])
                nc.gpsimd.collective_compute(
                    "AllReduce", mybir.AluOpType.add, replica_groups=rg,
                    ins=[pool_l.opt()], outs=[pool_g.opt()])
                if DEBUG_STAGE == "POOL":
                    nc.sync.dma_start(dbg[:], pool_g[:])

                # ---------------- MLP (replicated on every core) ----------------
                pg_sb = []
                for g, (glo, gsz) in enumerate(gsp):
                    t32 = small.tile([128, N_GRAPHS], F32, tag="pg32")
                    nc.sync.dma_start(t32[:gsz, :], pool_g[glo:glo + gsz, :])
                    t = persist.tile([128, N_GRAPHS], CDT, name=f"pg{g}")
                    nc.vector.tensor_copy(t[:gsz, :], t32[:gsz, :])
                    pg_sb.append(t)
                wm1_sb = []
                for g, (glo, gsz) in enumerate(gsp):
                    w = persist.tile([128, MLP_HID], CDT, name=f"wm1_{g}")
                    nc.sync.dma_start(w[:gsz, :], Wm1[glo:glo + gsz, :])
                    wm1_sb.append(w)
                msp = _chunks(MLP_HID)
                bm1_sb = []
                for m, (mlo, msz) in enumerate(msp):
                    b = persist.tile([128, 1], F32, name=f"bm1_{m}")
                    nc.sync.dma_start(b[:msz, :], bm1[mlo:mlo + msz, :])
                    bm1_sb.append(b)
                h1_sb = []
                for m, (mlo, msz) in enumerate(msp):
                    psm = ps.tile([128, N_GRAPHS], F32, tag="p7")
                    for g, (glo, gsz) in enumerate(gsp):
                        _mm(nc, psm[:msz, :], wm1_sb[g][:gsz, mlo:mlo + msz],
                            pg_sb[g][:gsz, :], start=(g == 0), stop=(g == len(gsp) - 1))
                    t = persist.tile([128, N_GRAPHS], CDT, name=f"h1_{m}")
                    nc.scalar.activation(t[:msz, :], psm[:msz, :], AF.Relu,
                                         bias=bm1_sb[m][:msz, :])
                    h1_sb.append(t)
                wm2_sb = []
                for m, (mlo, msz) in enumerate(msp):
                    w = persist.tile([128, NOUT], CDT, name=f"wm2_{m}")
                    nc.sync.dma_start(w[:msz, :], Wm2[mlo:mlo + msz, :])
                    wm2_sb.append(w)
                for o, (olo, osz) in enumerate(_chunks(NOUT)):
                    b2 = small.tile([128, 1], F32, tag="bm2")
                    nc.sync.dma_start(b2[:osz, :], bm2[olo:olo + osz, :])
                    psm = ps.tile([128, N_GRAPHS], F32, tag="p7")
                    for m, (mlo, msz) in enumerate(msp):
                        _mm(nc, psm[:osz, :], wm2_sb[m][:msz, olo:olo + osz],
                            h1_sb[m][:msz, :], start=(m == 0), stop=(m == len(msp) - 1))
                    ot = small.tile([128, N_GRAPHS], F32, tag="ot")
                    nc.vector.tensor_scalar_add(ot[:osz, :], psm[:osz, :], b2[:osz, :])
                    nc.sync.dma_start(outT[olo:olo + osz, :], ot[:osz, :])


            # ---------------- layers 0, 1 ----------------
            def dump_ht():
                for ci, (lo, sz) in enumerate(_chunks(HEADS * HID)):
                    tdb = rstream.tile([128, RPC], CDT, tag="tdb", name=f"tdb{ci}")
                    nc.sync.dma_start(tdb[:sz, :], hT[lo:lo + sz, :])
                    t32 = rstream.tile([128, RPC], F32, tag="tdb32", name=f"t32{ci}")
                    nc.vector.tensor_copy(t32[:sz, :], tdb[:sz, :])
                    nc.sync.dma_start(dbg[lo:lo + sz, :], t32[:sz, :])

            wg00 = gat_layer(0, F_IN, R0, WA0,
                      lambda t, lo, sz: nc.sync.dma_start(t[:sz, :], xT[lo:lo + sz, :]))
            stop = False
            if DEBUG_STAGE == "L0":
                dump_ht()
                stop = True
            elif DEBUG_STAGE == "S0":
                s32 = persist.tile([HEADS, RPC], F32, name="s32dbg")
                nc.vector.tensor_copy(s32[:], s_sb[:])
                nc.sync.dma_start(dbg[:], s32[:])
                stop = True
            elif DEBUG_STAGE == "WG0":
                for ci in range(NCH):
                    tdb = rstream.tile([128, HID + 1], CDT, tag="tdb", name=f"tdb{ci}")
                    nc.sync.dma_start(tdb[:], wg00[ci * 128:(ci + 1) * 128, :])
                    t32 = rstream.tile([128, HID + 1], F32, tag="tdb32", name=f"t32{ci}")
                    nc.vector.tensor_copy(t32[:], tdb[:])
                    nc.sync.dma_start(dbg[ci * 128:(ci + 1) * 128, :], t32[:])
                stop = True
            if not stop:
                gat_layer(1, HEADS * HID, R1, WA1,
                          lambda t, lo, sz: nc.sync.dma_start(t[:sz, :], hT[lo:lo + sz, :]))
                if DEBUG_STAGE == "L1":
                    dump_ht()
                    stop = True
            if not stop:
                _tail()

    nc.compile()
    return nc


# ======================= host side =======================

def _np_cdt(a):
    if CDT == F32:
        return np.ascontiguousarray(a, dtype=np.float32)
    import ml_dtypes
    return np.ascontiguousarray(np.asarray(a, np.float32).astype(ml_dtypes.bfloat16))


def _prep_inputs(x, edge_index, batch, W0, a0_src, a0_dst, W1, a1_src, a1_dst,
                 W_out, ao_src, ao_dst, Wm1, bm1, Wm2, bm2):
    x = np.asarray(x, np.float32)
    ei = np.asarray(edge_index)
    batch = np.asarray(batch).astype(np.int64)
    adj = np.zeros((N, N), np.float32)
    adj[ei[0], ei[1]] = 1.0

    cnt = np.bincount(batch, minlength=N_GRAPHS).astype(np.float32)
    cnt = np.maximum(cnt, 1.0)
    smat_full = np.zeros((N, N_GRAPHS), np.float32)
    smat_full[np.arange(N), batch] = 1.0 / cnt[batch]

    W0 = np.asarray(W0, np.float32)
    W1 = np.asarray(W1, np.float32)
    W_out = np.asarray(W_out, np.float32)

    def fuse(W, a_dst):  # [H, F, O], [H, O] -> [H, F, O+1]
        wad = np.einsum('hfo,ho->hf', W, np.asarray(a_dst, np.float32))
        return np.concatenate([W, wad[:, :, None]], axis=2)

    R0p = fuse(W0, a0_dst)
    WA0p = np.einsum('hfo,ho->fh', W0, np.asarray(a0_src, np.float32))
    R1p = fuse(W1, a1_dst)
    WA1p = np.einsum('hfo,ho->fh', W1, np.asarray(a1_src, np.float32))
    Rop = np.concatenate(
        [W_out, (W_out @ np.asarray(ao_dst, np.float32))[:, None]], axis=1)
    WAop = (W_out @ np.asarray(ao_src, np.float32))[:, None]

    shared = dict(
        R0=_np_cdt(R0p), WA0=_np_cdt(WA0p), R1=_np_cdt(R1p), WA1=_np_cdt(WA1p),
        Ro=_np_cdt(Rop), WAo=_np_cdt(WAop),
        Wm1=_np_cdt(Wm1), bm1=np.ascontiguousarray(np.asarray(bm1, np.float32)[:, None]),
        Wm2=_np_cdt(Wm2), bm2=np.ascontiguousarray(np.asarray(bm2, np.float32)[:, None]),
    )
    eye = np.kron(np.eye(HEADS, dtype=np.float32), np.ones((1, 128), np.float32))
    shared["eye10f"] = np.ascontiguousarray(eye)
    shared["eye10c"] = _np_cdt(eye)
    xT_full = x.T
    in_maps = []
    for c in range(NCORES):
        rows = slice(c * RPC, (c + 1) * RPC)
        m = dict(shared)
        m["xT"] = _np_cdt(xT_full[:, rows])
        m["adjT"] = _np_cdt(adj[rows, :].T)
        m["smat"] = _np_cdt(smat_full[rows, :])
        in_maps.append(m)
    return in_maps


_last_results = None


def kernel(**inputs):
    global _last_results
    if "k" not in _compiled:
        _compiled["k"] = build()
    nc = _compiled["k"]
    in_maps = _prep_inputs(**inputs)
    kw = {}
    if TRACE:
        import tracehook
        tracehook.install()
        kw = dict(trace=True)
        td = os.environ.get("KERNEL_TRACEDIR")
        if td:
            kw["tmpdir"] = td
    res = run_bass_kernel_spmd(nc, in_maps, core_ids=list(range(NCORES)), **kw)
    _last_results = res
    return np.ascontiguousarray(res.results[0]["outT"].T)



# revision 5
# speedup vs baseline: 1.0957x; 1.0957x over previous
"""Trainium2 Bass kernel for nn_DeepGATEncoder (3-layer GAT + mean-pool + MLP).

Sharding: node rows split 384/core across 8 cores; weights replicated.
Per GAT layer each core computes Wh (+ fused a_dst / a_src columns) for its
own 384 nodes for all 10 heads; the per-head [ones|Wh|d] blocks are
AllGather'ed in TWO batched collectives (heads 0-4, 5-9) to keep the
serialized collective chain short, then each core runs masked-softmax
attention for its own rows against all 3072 columns.

Attention matmuls are "flipped": stationary = 128x128 block of the masked
exp matrix p (j on partitions), moving = gathered [ones|Wh] chunk, so the
output lands as [i, o] with the softmax denominator in column 0 --
normalization is a per-partition scalar multiply. ELU'd outputs accumulate
in SBUF in [i, o] layout and one PE-transpose pass per layer rebuilds the
h^T chunk tiles for the next layer's matmuls; h never round-trips DRAM.

Softmax uses the overflow-safe identity
    exp(lrelu(z)) = max(exp(z), exp(.02 z)),  z = s_i + d_j
with s, d falling out of the Wh matmul via fused weight columns
(W@a_dst, W@a_src appended to W).
"""

import os
import numpy as np

import concourse.bass as bass
import concourse.bacc as bacc
import concourse.mybir as mybir
import concourse.tile as tile
from concourse.bass_utils import run_bass_kernel_spmd

# ---- problem constants (hardcoded; kernel.py must be self-contained) ----
N = 3072
F_IN = 300
HID = 300
OUT_ATT = 600
HEADS = 10
N_GRAPHS = 96
MLP_HID = 600
NOUT = 768
ALPHA = 0.02

NCORES = 8
RPC = N // NCORES          # 384 rows (nodes) per core
NJT = RPC // 128           # 3 own-row tiles of 128
NCH = N // 128             # 24 column chunks of 128
GRP = 3                    # chunks per elementwise group
HB = HEADS // 2            # heads per AllGather batch

W2 = HID + 2               # fused R columns: Wh(300) | d | s
GW = HID + 2               # gathered per-head width: ones | Wh(300) | d
SW = HID + 1               # streamed width: ones | Wh(300)
WO2 = OUT_ATT + 2          # fused Ro columns: Wh(600) | d | s
GWO = OUT_ATT + 2          # gathered: ones | Wh(600) | d
SWO = OUT_ATT + 1          # streamed: ones | Wh(600)

F32 = mybir.dt.float32
BF16 = mybir.dt.bfloat16
AF = mybir.ActivationFunctionType
CDT = BF16

TRACE = bool(os.environ.get("KERNEL_TRACE"))
DEBUG_STAGE = os.environ.get("KERNEL_DEBUG", "")

_compiled = {}


def _chunks(total, step=128):
    out = []
    lo = 0
    while lo < total:
        out.append((lo, min(step, total - lo)))
        lo += step
    return out


def _mm(nc, out, lhsT, rhs, **kw):
    if lhsT.dtype == F32:
        lhsT = lhsT.bitcast(mybir.dt.float32r)
        rhs = rhs.bitcast(mybir.dt.float32r)
    nc.tensor.matmul(out, lhsT, rhs, **kw)


def build():
    nc = bacc.Bacc("TRN2", target_bir_lowering=False, debug=False,
                   num_devices=NCORES)

    xT = nc.dram_tensor("xT", [F_IN, RPC], CDT, kind="ExternalInput")
    adjT = nc.dram_tensor("adjT", [N, RPC], CDT, kind="ExternalInput")
    smat = nc.dram_tensor("smat", [RPC, N_GRAPHS], CDT, kind="ExternalInput")
    R0 = nc.dram_tensor("R0", [HEADS, F_IN, W2], CDT, kind="ExternalInput")
    R1 = nc.dram_tensor("R1", [HEADS, HEADS * HID, W2], CDT, kind="ExternalInput")
    Ro = nc.dram_tensor("Ro", [HEADS * HID, WO2], CDT, kind="ExternalInput")
    Wm1 = nc.dram_tensor("Wm1", [OUT_ATT, MLP_HID], CDT, kind="ExternalInput")
    bm1 = nc.dram_tensor("bm1", [MLP_HID, 1], F32, kind="ExternalInput")
    Wm2 = nc.dram_tensor("Wm2", [MLP_HID, NOUT], CDT, kind="ExternalInput")
    bm2 = nc.dram_tensor("bm2", [NOUT, 1], F32, kind="ExternalInput")
    eye128 = nc.dram_tensor("eye128", [128, 128], CDT, kind="ExternalInput")
    outT = nc.dram_tensor("outT", [NOUT, N_GRAPHS], F32, kind="ExternalOutput")
    dbg = None
    if DEBUG_STAGE in ("L0", "L1"):
        dbg = nc.dram_tensor("dbg", [HEADS * HID, RPC], F32, kind="ExternalOutput")
    elif DEBUG_STAGE == "HO":
        dbg = nc.dram_tensor("dbg", [RPC, OUT_ATT], F32, kind="ExternalOutput")
    elif DEBUG_STAGE == "POOL":
        dbg = nc.dram_tensor("dbg", [OUT_ATT, N_GRAPHS], F32, kind="ExternalOutput")

    rg = [list(range(NCORES))]
    FO = HEADS * HID
    kch_o = _chunks(FO)        # 23x128 + 56

    with tile.TileContext(nc) as tc:
        with (
            tc.tile_pool(name="persist", bufs=1) as persist,
            tc.tile_pool(name="whbuf", bufs=2) as whbufp,
            tc.tile_pool(name="rstream", bufs=4) as rstream,
            tc.tile_pool(name="ew", bufs=3) as ew,
            tc.tile_pool(name="small", bufs=2) as small,
            tc.tile_pool(name="ps", bufs=1, space="PSUM") as ps,
            tc.tile_pool(name="dram", bufs=1, space="DRAM") as dram,
        ):
            # ---------- persistent SBUF state ----------
            adj_sb = persist.tile([128, NCH * RPC], CDT, name="adj_sb")
            nc.sync.dma_start(adj_sb[:].rearrange("p (c i) -> p c i", i=RPC),
                              adjT[:].rearrange("(c p) i -> p c i", p=128))
            smat_sb = [persist.tile([128, N_GRAPHS], CDT, name=f"smat{i}")
                       for i in range(NJT)]
            for i in range(NJT):
                nc.sync.dma_start(smat_sb[i][:], smat[i * 128:(i + 1) * 128, :])
            eye_sb = persist.tile([128, 128], CDT, name="eye_sb")
            nc.sync.dma_start(eye_sb[:], eye128[:])
            onesf_sb = persist.tile([1, 128], F32, name="onesf_sb")
            nc.vector.memset(onesf_sb[:], 1.0)
            onesc_sb = persist.tile([1, 128], CDT, name="onesc_sb")
            nc.vector.memset(onesc_sb[:], 1.0)

            # MLP weights prefetched up-front on the scalar ring
            gsp = _chunks(OUT_ATT)
            msp = _chunks(MLP_HID)
            wm1_sb = []
            for g, (glo, gsz) in enumerate(gsp):
                w = persist.tile([128, MLP_HID], CDT, name=f"wm1_{g}")
                nc.scalar.dma_start(w[:gsz, :], Wm1[glo:glo + gsz, :])
                wm1_sb.append(w)
            wm2_sb = []
            for m, (mlo, msz) in enumerate(msp):
                w = persist.tile([128, NOUT], CDT, name=f"wm2_{m}")
                nc.scalar.dma_start(w[:msz, :], Wm2[mlo:mlo + msz, :])
                wm2_sb.append(w)
            bm1_sb = []
            for m, (mlo, msz) in enumerate(msp):
                b = persist.tile([128, 1], F32, name=f"bm1_{m}")
                nc.scalar.dma_start(b[:msz, :], bm1[mlo:mlo + msz, :])
                bm1_sb.append(b)

            # h^T chunk tiles (next-layer matmul inputs) + h in [i, o] layout
            ht = [persist.tile([128, RPC], CDT, name=f"ht{kc}")
                  for kc in range(len(kch_o))]
            hfull = [persist.tile([128, FO], CDT, name=f"hfull{t}")
                     for t in range(NJT)]

            # broadcast tiles per head (held for the whole layer)
            sbc_all = persist.tile([128, HEADS * RPC], F32, name="sbc_all")
            sexp_all = persist.tile([128, HEADS * RPC], CDT, name="sexp_all")
            e02bc_all = persist.tile([128, HEADS * RPC], CDT, name="e02bc_all")

            # layer-0 input h^T = xT
            for ci, (lo, sz) in enumerate(_chunks(F_IN)):
                nc.sync.dma_start(ht[ci][:sz, :], xT[lo:lo + sz, :])

            def s_transpose_and_bcast(wl_sb, stride, hslot):
                """PE part of the per-head s handling: transpose the s
                columns (at offset stride-1 within each jt block of wl_sb)
                into a [1, RPC] row, then broadcast s / exp(s) / exp(.02 s)
                down 128 partitions into the *_all tiles at hslot."""
                pst = ps.tile([1, RPC], F32, tag="p6", name="pst")
                for jt in range(NJT):
                    base = jt * stride
                    nc.tensor.matmul(pst[:, jt * 128:(jt + 1) * 128],
                                     wl_sb[:, base + stride - 1:base + stride],
                                     eye_sb[:], start=True, stop=True)
                cs = slice(hslot * RPC, (hslot + 1) * RPC)
                s_row = small.tile([1, RPC], F32, tag="s_row")
                nc.scalar.activation(s_row[:], pst[:], AF.Copy)
                se_row = small.tile([1, RPC], CDT, tag="se_row")
                nc.scalar.activation(se_row[:], pst[:], AF.Exp)
                e2_row = small.tile([1, RPC], CDT, tag="e2_row")
                nc.scalar.activation(e2_row[:], pst[:], AF.Exp, scale=ALPHA)
                pb = ps.tile([128, RPC], F32, tag="p7", name="pb")
                nc.tensor.matmul(pb[:], onesf_sb[:], s_row[:],
                                 start=True, stop=True)
                nc.scalar.activation(sbc_all[:, cs], pb[:], AF.Copy)
                pb2 = ps.tile([128, RPC], F32, tag="p7", name="pb2")
                nc.tensor.matmul(pb2[:], onesc_sb[:], se_row[:],
                                 start=True, stop=True)
                nc.scalar.activation(sexp_all[:, cs], pb2[:], AF.Copy)
                pb3 = ps.tile([128, RPC], F32, tag="p7", name="pb3")
                nc.tensor.matmul(pb3[:], onesc_sb[:], e2_row[:],
                                 start=True, stop=True)
                nc.scalar.activation(e02bc_all[:, cs], pb3[:], AF.Copy)

            # ============ one multi-head GAT layer ============
            def gat_layer(lidx, fin, r_dram):
                kch = _chunks(fin)
                nkc = len(kch)
                wl = [dram.tile([RPC, HB * GW], CDT, name=f"wl{lidx}_{b}")
                      for b in range(2)]
                wg = [dram.tile([N, HB * GW], CDT, name=f"wg{lidx}_{b}",
                                addr_space="Shared") for b in range(2)]

                # --- Wh phase; AG after heads HB-1 and HEADS-1 ---
                stride = W2 + 1        # per-jt block in wl_sb: ones|Wh|d|s
                pend = None
                for h in range(HEADS):
                    b, h5 = h // HB, h % HB
                    psw = [ps.tile([128, W2], F32, tag=f"p{jt}",
                                   name=f"psw{jt}") for jt in range(NJT)]
                    for ci, (lo, sz) in enumerate(kch):
                        r_t = rstream.tile([128, W2], CDT, tag="r", bufs=12)
                        nc.sync.dma_start(r_t[:sz, :], r_dram[h, lo:lo + sz, :])
                        for jt in range(NJT):
                            _mm(nc, psw[jt][:],
                                ht[ci][:sz, jt * 128:(jt + 1) * 128],
                                r_t[:sz, :],
                                start=(ci == 0), stop=(ci == nkc - 1))
                    # wl_sb per jt: [ones | Wh(300) | d | s]
                    wl_sb = small.tile([128, NJT * stride], CDT, tag="wl_sb")
                    for jt in range(NJT):
                        base = jt * stride
                        nc.vector.memset(wl_sb[:, base:base + 1], 1.0)
                        nc.scalar.activation(wl_sb[:, base + 1:base + stride],
                                             psw[jt][:], AF.Copy)
                        nc.scalar.dma_start(
                            wl[b][jt * 128:(jt + 1) * 128,
                                  h5 * GW:(h5 + 1) * GW],
                            wl_sb[:, base:base + GW])
                    # defer the PE s-transpose/broadcast by one head so its
                    # ACT deps never bubble the PE queue
                    if pend is not None:
                        s_transpose_and_bcast(*pend)
                    pend = (wl_sb, stride, h)
                    if h5 == HB - 1:
                        nc.gpsimd.collective_compute(
                            "AllGather", mybir.AluOpType.bypass,
                            replica_groups=rg, ins=[wl[b].opt()],
                            outs=[wg[b].opt()])
                s_transpose_and_bcast(*pend)

                # --- attention phase ---
                for h in range(HEADS):
                    b, h5 = h // HB, h % HB
                    cs = slice(h * RPC, (h + 1) * RPC)
                    whb = whbufp.tile([128, NCH * GW], CDT, tag="whb",
                                      name=f"whb{h}")
                    nc.scalar.dma_start(
                        whb[:].rearrange("p (c w) -> p c w", w=GW),
                        wg[b][:, h5 * GW:(h5 + 1) * GW]
                        .rearrange("(c p) w -> p c w", p=128))
                    dcols = whb[:].rearrange("p (c w) -> p c w", w=GW)[:, :, GW - 1]
                    ed = small.tile([128, NCH], F32, tag="ed", name=f"ed{h}")
                    nc.scalar.activation(ed[:], dcols, AF.Exp)
                    ed02 = small.tile([128, NCH], F32, tag="ed02",
                                      name=f"ed02_{h}")
                    nc.scalar.activation(ed02[:], dcols, AF.Exp, scale=ALPHA)

                    pa = [ps.tile([128, SW], F32, tag=f"p{(h % 2) * 3 + t}",
                                  name=f"pa{t}") for t in range(NJT)]
                    for c0 in range(0, NCH, GRP):
                        a_t = ew.tile([128, GRP * RPC], CDT, tag="a", bufs=2)
                        b_t = ew.tile([128, GRP * RPC], CDT, tag="b", bufs=2)
                        for k in range(GRP):
                            c = c0 + k
                            ks = slice(k * RPC, (k + 1) * RPC)
                            if c % 3 == 0:
                                nc.vector.tensor_scalar_mul(
                                    a_t[:, ks], sexp_all[:, cs], ed[:, c:c + 1])
                            else:
                                nc.scalar.activation(
                                    a_t[:, ks], sbc_all[:, cs], AF.Exp,
                                    bias=whb[:, c * GW + GW - 1:c * GW + GW])
                            nc.vector.tensor_scalar_mul(
                                b_t[:, ks], e02bc_all[:, cs], ed02[:, c:c + 1])
                        nc.vector.tensor_max(b_t[:], a_t[:], b_t[:])
                        p_t = ew.tile([128, GRP * RPC], CDT, tag="p", bufs=3)
                        nc.vector.tensor_mul(
                            p_t[:], b_t[:], adj_sb[:, c0 * RPC:(c0 + GRP) * RPC])
                        for k in range(GRP):
                            c = c0 + k
                            for t in range(NJT):
                                _mm(nc, pa[t][:],
                                    p_t[:, k * RPC + t * 128:k * RPC + (t + 1) * 128],
                                    whb[:, c * GW:c * GW + SW],
                                    start=(c == 0), stop=(c == NCH - 1))
                    # normalize + ELU straight into hfull
                    for t in range(NJT):
                        den = small.tile([128, 1], F32, tag="den")
                        scr = small.tile([128, 1], F32, tag="scr")
                        rden = small.tile([128, 1], F32, tag="rden")
                        nc.vector.tensor_copy(den[:], pa[t][:, 0:1])
                        nc.vector.reciprocal_approx_accurate(rden[:], den[:],
                                                             scr[:])
                        y = hfull[t][:, h * HID:(h + 1) * HID]
                        nc.vector.tensor_scalar_mul(y, pa[t][:, 1:SW], rden[:])
                        q = ew.tile([128, HID], CDT, tag="q")
                        nc.scalar.activation(q[:], y, AF.Exp)
                        nc.vector.tensor_scalar(q[:], q[:], -1.0, 0.0,
                                                mybir.AluOpType.add,
                                                mybir.AluOpType.min)
                        nc.vector.tensor_max(y, y, q[:])

                # --- transpose h [i, o] -> h^T chunk tiles ---
                for kc, (lo, sz) in enumerate(kch_o):
                    psT = ps.tile([128, RPC], F32, tag=f"p{6 + kc % 2}",
                                  name=f"psT{kc}")
                    for t in range(NJT):
                        _mm(nc, psT[:sz, t * 128:(t + 1) * 128],
                            hfull[t][:, lo:lo + sz], eye_sb[:],
                            start=True, stop=True)
                    nc.scalar.activation(ht[kc][:sz, :], psT[:sz, :], AF.Copy)

            # ---------------- output attention layer + pool + MLP ----------
            def _tail():
                nkc = len(kch_o)
                wlo = dram.tile([RPC, GWO], CDT, name="wlo")
                wgo = dram.tile([N, GWO], CDT, name="wgo", addr_space="Shared")
                nsp = [(0, 512), (512, WO2 - 512)]
                stride = WO2 + 1       # ones|Wh(600)|d|s
                wlo_sb = small.tile([128, NJT * stride], CDT, tag="wlo_sb")
                for jt in range(NJT):
                    pswo = [ps.tile([128, sz], F32, tag=f"p{jt * 2 + si}",
                                    name=f"pswo{si}")
                            for si, (lo, sz) in enumerate(nsp)]
                    for ci, (lo, sz) in enumerate(kch_o):
                        r_t = rstream.tile([128, WO2], CDT, tag="ro", bufs=6)
                        nc.sync.dma_start(r_t[:sz, :], Ro[lo:lo + sz, :])
                        for si, (slo, ssz) in enumerate(nsp):
                            _mm(nc, pswo[si][:],
                                ht[ci][:sz, jt * 128:(jt + 1) * 128],
                                r_t[:sz, slo:slo + ssz],
                                start=(ci == 0), stop=(ci == nkc - 1))
                    base = jt * stride
                    nc.vector.memset(wlo_sb[:, base:base + 1], 1.0)
                    for si, (slo, ssz) in enumerate(nsp):
                        nc.scalar.activation(
                            wlo_sb[:, base + 1 + slo:base + 1 + slo + ssz],
                            pswo[si][:], AF.Copy)
                    nc.scalar.dma_start(wlo[jt * 128:(jt + 1) * 128, :],
                                        wlo_sb[:, base:base + GWO])
                s_transpose_and_bcast(wlo_sb, stride, 0)
                nc.gpsimd.collective_compute(
                    "AllGather", mybir.AluOpType.bypass, replica_groups=rg,
                    ins=[wlo.opt()], outs=[wgo.opt()])

                cs = slice(0, RPC)
                onspl = [(0, 512), (512, SWO - 512)]   # slices of ones|Wh
                ps_o = [[ps.tile([128, sz], F32, tag=f"p{t * 2 + si}",
                                 name=f"pso{t}_{si}")
                         for si, (lo, sz) in enumerate(onspl)]
                        for t in range(NJT)]
                for c0 in range(0, NCH, GRP):
                    whcs = []
                    a_t = ew.tile([128, GRP * RPC], CDT, tag="a", bufs=2)
                    b_t = ew.tile([128, GRP * RPC], CDT, tag="b", bufs=2)
                    for k in range(GRP):
                        c = c0 + k
                        ks = slice(k * RPC, (k + 1) * RPC)
                        whc = rstream.tile([128, GWO], CDT, tag=f"whc{k}",
                                           name=f"whc{c}", bufs=2)
                        nc.scalar.dma_start(whc[:],
                                            wgo[c * 128:(c + 1) * 128, :])
                        whcs.append(whc)
                        edc = small.tile([128, 1], F32, tag="edc", bufs=3)
                        nc.scalar.activation(edc[:], whc[:, GWO - 1:GWO],
                                             AF.Exp)
                        ed02c = small.tile([128, 1], F32, tag="ed02c", bufs=3)
                        nc.scalar.activation(ed02c[:], whc[:, GWO - 1:GWO],
                                             AF.Exp, scale=ALPHA)
                        if c % 3 == 0:
                            nc.vector.tensor_scalar_mul(
                                a_t[:, ks], sexp_all[:, cs], edc[:])
                        else:
                            nc.scalar.activation(
                                a_t[:, ks], sbc_all[:, cs], AF.Exp,
                                bias=whc[:, GWO - 1:GWO])
                        nc.vector.tensor_scalar_mul(
                            b_t[:, ks], e02bc_all[:, cs], ed02c[:])
                    nc.vector.tensor_max(b_t[:], a_t[:], b_t[:])
                    p_t = ew.tile([128, GRP * RPC], CDT, tag="p", bufs=3)
                    nc.vector.tensor_mul(p_t[:], b_t[:],
                                         adj_sb[:, c0 * RPC:(c0 + GRP) * RPC])
                    for k in range(GRP):
                        c = c0 + k
                        for t in range(NJT):
                            for si, (slo, ssz) in enumerate(onspl):
                                _mm(nc, ps_o[t][si][:],
                                    p_t[:, k * RPC + t * 128:k * RPC + (t + 1) * 128],
                                    whcs[k][:, slo:slo + ssz],
                                    start=(c == 0), stop=(c == NCH - 1))

                # normalize + ELU; att_tiles[t]: [128, 600]
                att_tiles = []
                for t in range(NJT):
                    deno = small.tile([128, 1], F32, tag="deno")
                    scro = small.tile([128, 1], F32, tag="scro")
                    rdeno = small.tile([128, 1], F32, tag="rdeno")
                    nc.vector.tensor_copy(deno[:], ps_o[t][0][:, 0:1])
                    nc.vector.reciprocal_approx_accurate(rdeno[:], deno[:],
                                                         scro[:])
                    att_sb = ew.tile([128, OUT_ATT], CDT, tag=f"atts{t}",
                                     name=f"atts{t}", bufs=1)
                    nc.vector.tensor_scalar_mul(att_sb[:, 0:511],
                                                ps_o[t][0][:, 1:512], rdeno[:])
                    nc.vector.tensor_scalar_mul(att_sb[:, 511:OUT_ATT],
                                                ps_o[t][1][:, 0:SWO - 512],
                                                rdeno[:])
                    q = ew.tile([128, OUT_ATT], CDT, tag="qo")
                    nc.scalar.activation(q[:], att_sb[:], AF.Exp)
                    nc.vector.tensor_scalar(q[:], q[:], -1.0, 0.0,
                                            mybir.AluOpType.add,
                                            mybir.AluOpType.min)
                    nc.vector.tensor_max(att_sb[:], att_sb[:], q[:])
                    if DEBUG_STAGE == "HO":
                        a32 = ew.tile([128, OUT_ATT], F32, tag="a32dbg")
                        nc.vector.tensor_copy(a32[:], att_sb[:])
                        nc.sync.dma_start(dbg[t * 128:(t + 1) * 128, :], a32[:])
                    att_tiles.append(att_sb)

                # pooled^T [600, 96] -> AllReduce
                pool_l = dram.tile([OUT_ATT, N_GRAPHS], F32, name="pool_l")
                pool_g = dram.tile([OUT_ATT, N_GRAPHS], F32, name="pool_g",
                                   addr_space="Shared")
                for g, (glo, gsz) in enumerate(gsp):
                    psp = ps.tile([128, N_GRAPHS], F32, tag=f"p{6 + g % 2}",
                                  name=f"psp{g}")
                    for t in range(NJT):
                        _mm(nc, psp[:gsz, :], att_tiles[t][:, glo:glo + gsz],
                            smat_sb[t][:], start=(t == 0), stop=(t == NJT - 1))
                    pl_sb = small.tile([128, N_GRAPHS], F32, tag="pl_sb")
                    nc.scalar.activation(pl_sb[:gsz, :], psp[:gsz, :], AF.Copy)
                    nc.sync.dma_start(pool_l[glo:glo + gsz, :], pl_sb[:gsz, :])
                nc.gpsimd.collective_compute(
                    "AllReduce", mybir.AluOpType.add, replica_groups=rg,
                    ins=[pool_l.opt()], outs=[pool_g.opt()])
                if DEBUG_STAGE == "POOL":
                    nc.sync.dma_start(dbg[:], pool_g[:])

                # ---------------- MLP (replicated) ----------------
                pg_sb = []
                for g, (glo, gsz) in enumerate(gsp):
                    t32 = small.tile([128, N_GRAPHS], F32, tag="pg32")
                    nc.sync.dma_start(t32[:gsz, :], pool_g[glo:glo + gsz, :])
                    t = persist.tile([128, N_GRAPHS], CDT, name=f"pg{g}")
                    nc.vector.tensor_copy(t[:gsz, :], t32[:gsz, :])
                    pg_sb.append(t)
                h1_sb = []
                for m, (mlo, msz) in enumerate(msp):
                    psm = ps.tile([128, N_GRAPHS], F32, tag=f"p{m % 2}")
                    for g, (glo, gsz) in enumerate(gsp):
                        _mm(nc, psm[:msz, :], wm1_sb[g][:gsz, mlo:mlo + msz],
                            pg_sb[g][:gsz, :], start=(g == 0),
                            stop=(g == len(gsp) - 1))
                    t = persist.tile([128, N_GRAPHS], CDT, name=f"h1_{m}")
                    nc.scalar.activation(t[:msz, :], psm[:msz, :], AF.Relu,
                                         bias=bm1_sb[m][:msz, :])
                    h1_sb.append(t)
                for o, (olo, osz) in enumerate(_chunks(NOUT)):
                    b2 = small.tile([128, 1], F32, tag="bm2")
                    nc.sync.dma_start(b2[:osz, :], bm2[olo:olo + osz, :])
                    psm = ps.tile([128, N_GRAPHS], F32, tag=f"p{2 + o % 2}")
                    for m, (mlo, msz) in enumerate(msp):
                        _mm(nc, psm[:osz, :], wm2_sb[m][:msz, olo:olo + osz],
                            h1_sb[m][:msz, :], start=(m == 0),
                            stop=(m == len(msp) - 1))
                    ot = small.tile([128, N_GRAPHS], F32, tag="ot")
                    nc.vector.tensor_scalar_add(ot[:osz, :], psm[:osz, :],
                                                b2[:osz, :])
                    nc.sync.dma_start(outT[olo:olo + osz, :], ot[:osz, :])

            # ---------------- run the stages ----------------
            def dump_ht():
                for kc, (lo, sz) in enumerate(kch_o):
                    t32 = rstream.tile([128, RPC], F32, tag="tdb32",
                                       name=f"t32{kc}")
                    nc.vector.tensor_copy(t32[:sz, :], ht[kc][:sz, :])
                    nc.sync.dma_start(dbg[lo:lo + sz, :], t32[:sz, :])

            gat_layer(0, F_IN, R0)
            stop = False
            if DEBUG_STAGE == "L0":
                dump_ht()
                stop = True
            if not stop:
                gat_layer(1, FO, R1)
                if DEBUG_STAGE == "L1":
                    dump_ht()
                    stop = True
            if not stop:
                _tail()

    nc.compile()
    return nc


# ======================= host side =======================

def _np_cdt(a):
    import ml_dtypes
    return np.ascontiguousarray(np.asarray(a, np.float32).astype(ml_dtypes.bfloat16))


def _prep_inputs(x, edge_index, batch, W0, a0_src, a0_dst, W1, a1_src, a1_dst,
                 W_out, ao_src, ao_dst, Wm1, bm1, Wm2, bm2):
    x = np.asarray(x, np.float32)
    ei = np.asarray(edge_index)
    batch = np.asarray(batch).astype(np.int64)
    adj = np.zeros((N, N), np.float32)
    adj[ei[0], ei[1]] = 1.0

    cnt = np.bincount(batch, minlength=N_GRAPHS).astype(np.float32)
    cnt = np.maximum(cnt, 1.0)
    smat_full = np.zeros((N, N_GRAPHS), np.float32)
    smat_full[np.arange(N), batch] = 1.0 / cnt[batch]

    W0 = np.asarray(W0, np.float32)
    W1 = np.asarray(W1, np.float32)
    W_out = np.asarray(W_out, np.float32)

    def fuse(W, a_dst, a_src):   # [H,F,O],[H,O],[H,O] -> [H,F,O+2]
        wad = np.einsum('hfo,ho->hf', W, np.asarray(a_dst, np.float32))
        was = np.einsum('hfo,ho->hf', W, np.asarray(a_src, np.float32))
        return np.concatenate([W, wad[:, :, None], was[:, :, None]], axis=2)

    R0p = fuse(W0, a0_dst, a0_src)
    R1p = fuse(W1, a1_dst, a1_src)
    Rop = np.concatenate(
        [W_out, (W_out @ np.asarray(ao_dst, np.float32))[:, None],
         (W_out @ np.asarray(ao_src, np.float32))[:, None]], axis=1)

    shared = dict(
        R0=_np_cdt(R0p), R1=_np_cdt(R1p), Ro=_np_cdt(Rop),
        Wm1=_np_cdt(Wm1),
        bm1=np.ascontiguousarray(np.asarray(bm1, np.float32)[:, None]),
        Wm2=_np_cdt(Wm2),
        bm2=np.ascontiguousarray(np.asarray(bm2, np.float32)[:, None]),
        eye128=_np_cdt(np.eye(128, dtype=np.float32)),
    )
    xT_full = x.T
    in_maps = []
    for c in range(NCORES):
        rows = slice(c * RPC, (c + 1) * RPC)
        m = dict(shared)
        m["xT"] = _np_cdt(xT_full[:, rows])
        m["adjT"] = _np_cdt(adj[rows, :].T)
        m["smat"] = _np_cdt(smat_full[rows, :])
        in_maps.append(m)
    return in_maps


_last_results = None


def kernel(**inputs):
    global _last_results
    if "k" not in _compiled:
        _compiled["k"] = build()
    nc = _compiled["k"]
    in_maps = _prep_inputs(**inputs)
    kw = {}
    if TRACE:
        import tracehook
        tracehook.install()
        kw = dict(trace=True)
        td = os.environ.get("KERNEL_TRACEDIR")
        if td:
            kw["tmpdir"] = td
    res = run_bass_kernel_spmd(nc, in_maps, core_ids=list(range(NCORES)), **kw)
    _last_results = res
    return np.ascontiguousarray(res.results[0]["outT"].T)
